# revision 17
# baseline (speedup 1.0000x reference)
"""Bass/Trainium2 kernel for nn_Bipartite_44014824849868 (GNN message passing).

Computes, for a full bipartite agent<-task graph:
  policy    = softmax_j( leaky_relu(ag_nfs @ w_att[D:] + task_nfs @ w_att[:D], 0.01) )  [n_ag, n_task]
  ag_policy = softmax( leaky_relu(ag_nfs @ w_ag, 0.01) )                                 [n_ag]

Sharding: agent (row) dimension split across 8 NeuronCores (256 rows each).
s_task = task_nfs @ w_att[:D] is computed sharded (2048 tasks per core) and
exchanged with one small AllGather; ag_policy is computed redundantly on
every core.
"""

import numpy as np

D = 128
N_AG = 2048
N_TASK = 16384
N_CORES = 8
R = N_AG // N_CORES          # 256 agent rows per core
T_LOC = N_TASK // N_CORES    # 2048 task rows per core
NEG = 0.01                   # leaky relu slope
CHUNK = 512                  # PSUM bank width in fp32
PIECE = 4096                 # output piece width (pipelining granularity)
NCH = N_TASK // CHUNK        # 32
NPIECE = N_TASK // PIECE     # 4
CH_PER_PIECE = PIECE // CHUNK

_CACHE = {}


def _build_program(_tlsim=False):
    import concourse.bacc as bacc
    import concourse.bass as bass
    import concourse.mybir as mybir
    from concourse import tile

    f32 = mybir.dt.float32
    AF = mybir.ActivationFunctionType
    ALU = mybir.AluOpType

    nc = bacc.Bacc("TRN2", target_bir_lowering=False, debug=False,
                   num_devices=1 if _tlsim else N_CORES)

    # ---- external I/O ----
    task_loc = nc.declare_dram_parameter("task_loc", [T_LOC, D], f32, isOutput=False)
    ag_full = nc.declare_dram_parameter("ag_full", [N_AG, D], f32, isOutput=False)
    ag_loc = nc.declare_dram_parameter("ag_loc", [R, D], f32, isOutput=False)
    w1b = nc.declare_dram_parameter("w1b", [128, D], f32, isOutput=False)
    w2b = nc.declare_dram_parameter("w2b", [128, D], f32, isOutput=False)
    wagb = nc.declare_dram_parameter("wagb", [128, D], f32, isOutput=False)
    ident = nc.declare_dram_parameter("ident", [128, 128], f32, isOutput=False)
    mask_cols = nc.declare_dram_parameter("mask_cols", [128, T_LOC // 128], f32,
                                          isOutput=False)
    policy_out = nc.declare_dram_parameter("policy_out", [R, N_TASK], f32,
                                           isOutput=True)
    agp_out = nc.declare_dram_parameter("agp_out", [N_AG // 128, 128], f32,
                                        isOutput=True)

    # ---- internal DRAM for the s_task exchange (bf16 hi/lo split) ----
    bf16 = mybir.dt.bfloat16
    st_loc_d = nc.dram_tensor("st_loc_d", [2, T_LOC // 128, 128], bf16)
    st_all_d = nc.dram_tensor("st_all_d", [N_CORES, 2, T_LOC], bf16,
                              addr_space="Shared")

    TT = T_LOC // 128   # 16 task tiles per core
    TA = N_AG // 128    # 16 agent tiles
    TL = R // 128       # 2 local agent tiles (= strips)

    with tile.TileContext(nc) as tc:
        with (
            tc.tile_pool(name="cpool", bufs=1) as cpool,
            tc.tile_pool(name="iop", bufs=2) as iop,
            tc.tile_pool(name="tscrp", bufs=3) as tscrp,
            tc.tile_pool(name="epool", bufs=5) as epool,
            tc.tile_pool(name="psmm", bufs=2, space=bass.MemorySpace.PSUM) as psmm,
        ):
            # ---------- input loads ----------
            task_sb = cpool.tile([128, TT, D], f32)
            task_view = task_loc[:, :].rearrange("(t p) d -> p t d", p=128)
            for q in range(4):
                nc.sync.dma_start(task_sb[:, q * (TT // 4):(q + 1) * (TT // 4), :],
                                  task_view[:, q * (TT // 4):(q + 1) * (TT // 4), :])
            ag_sb = cpool.tile([128, TA, D], f32)
            nc.sync.dma_start(
                ag_sb[:], ag_full[:, :].rearrange("(t p) d -> p t d", p=128))
            agl_sb = cpool.tile([128, TL, D], f32)
            nc.sync.dma_start(
                agl_sb[:], ag_loc[:, :].rearrange("(t p) d -> p t d", p=128))
            w1s = cpool.tile([128, D], f32)
            nc.sync.dma_start(w1s[:], w1b[:, :])
            w2s = cpool.tile([128, D], f32)
            nc.sync.dma_start(w2s[:], w2b[:, :])
            wags = cpool.tile([128, D], f32)
            nc.sync.dma_start(wags[:], wagb[:, :])
            idn = cpool.tile([128, 128], f32)
            nc.sync.dma_start(idn[:], ident[:, :])
            maskc = cpool.tile([128, TT], f32)
            nc.sync.dma_start(maskc[:], mask_cols[:, :])

            # ---------- matvecs (DVE fused mult+reduce) ----------
            st_cols = cpool.tile([128, TT], f32)      # s_task local, col layout
            for t in range(TT):
                scr = iop.tile([128, D], f32, tag="scr")
                nc.vector.tensor_mul(scr[:], task_sb[:, t, :], w1s[:])
                nc.vector.tensor_reduce(st_cols[:, t:t + 1], scr[:],
                                        axis=mybir.AxisListType.X, op=ALU.add)
            # fold task_finished mask into s_task (adds -1e30 on finished cols)
            nc.vector.tensor_add(st_cols[:], st_cols[:], maskc[:])

            # ---------- s_task: cols -> row order, bf16 hi/lo, AllGather ----------
            pst = psmm.tile([TT, 128], f32, tag="mm")
            nc.tensor.transpose(pst[:], st_cols[:], idn[:])
            stT = cpool.tile([TT, 128], f32)
            nc.scalar.copy(stT[:], pst[:])
            # hi/lo split so the PE can consume s_task at bf16 speed with
            # ~fp32 accuracy: s_task ~= hi + lo, both bf16
            st_hi = cpool.tile([TT, 128], bf16)
            nc.vector.tensor_copy(st_hi[:], stT[:])
            st_hi32 = cpool.tile([TT, 128], f32)
            nc.vector.tensor_copy(st_hi32[:], st_hi[:])
            st_lo32 = cpool.tile([TT, 128], f32)
            nc.vector.tensor_sub(st_lo32[:], stT[:], st_hi32[:])
            st_lo = cpool.tile([TT, 128], bf16)
            nc.vector.tensor_copy(st_lo[:], st_lo32[:])
            nc.sync.dma_start(st_loc_d[0, :, :], st_hi[:])
            nc.sync.dma_start(st_loc_d[1, :, :], st_lo[:])
            if _tlsim:
                # stand-in for the AllGather so TimelineSim (single core,
                # no collectives) can model the rest of the kernel
                for c_ in range(N_CORES):
                    nc.sync.dma_start(
                        st_all_d[c_, :, :].rearrange("h (t p) -> h t p", p=128),
                        st_loc_d[:, :, :])
            else:
                nc.gpsimd.collective_compute(
                    "AllGather", ALU.bypass,
                    replica_groups=[list(range(N_CORES))],
                    ins=[st_loc_d[:, :, :].rearrange("a b c -> (a b c)")],
                    outs=[st_all_d[:, :, :].rearrange("a b c -> (a b c)")],
                )
            # gathered layout: [core, hi/lo, 2048] -> rhs rows [2, 16384] bf16
            # (partition 0 = s_task_hi, partition 1 = s_task_lo), one DMA
            rhs_bf = cpool.tile([2, N_TASK], bf16)
            nc.sync.dma_start(
                rhs_bf[:, :].rearrange("h (c j) -> h c j", c=N_CORES),
                st_all_d[:, :, :].rearrange("c h j -> h c j"))

            sag_cols = cpool.tile([128, TL], f32)     # s_ag for local strips
            for t in range(TL):
                scr = iop.tile([128, D], f32, tag="scr")
                nc.vector.tensor_mul(scr[:], agl_sb[:, t, :], w2s[:])
                nc.vector.tensor_reduce(sag_cols[:, t:t + 1], scr[:],
                                        axis=mybir.AxisListType.X, op=ALU.add)

            ags_cols = cpool.tile([128, TA], f32)     # ag_score pre-act, all agents
            for t in range(TA):
                scr = iop.tile([128, D], f32, tag="scr")
                nc.vector.tensor_mul(scr[:], ag_sb[:, t, :], wags[:])
                nc.vector.tensor_reduce(ags_cols[:, t:t + 1], scr[:],
                                        axis=mybir.AxisListType.X, op=ALU.add)

            ones2_bf = cpool.tile([2, 128], bf16)
            nc.vector.memset(ones2_bf[:], 1.0)
            ones_r = cpool.tile([1, 128], f32)
            nc.vector.memset(ones_r[:], 1.0)
            ones_c = cpool.tile([128, 1], f32)
            nc.vector.memset(ones_c[:], 1.0)



            # ---------- ag_policy (redundant on every core) ----------
            agt = iop.tile([128, TA], f32, tag="agt")
            nc.scalar.activation(agt[:], ags_cols[:], AF.Prelu, alpha=NEG)
            age = iop.tile([128, TA], f32, tag="age")
            agz = iop.tile([128, 1], f32, tag="agz")
            nc.scalar.activation(age[:], agt[:], AF.Exp, accum_out=agz[:])
            psz = psmm.tile([1, 1], f32, tag="mm")
            nc.tensor.matmul(psz[:], agz[:], ones_c[:], start=True, stop=True)
            zz = iop.tile([1, 1], f32, tag="zz")
            nc.scalar.copy(zz[:], psz[:])
            psb = psmm.tile([128, 1], f32, tag="mm")
            nc.tensor.matmul(psb[:], ones_r[:], zz[:], start=True, stop=True)
            zb = iop.tile([128, 1], f32, tag="zb")
            nc.scalar.copy(zb[:], psb[:])
            rag = iop.tile([128, 1], f32, tag="rag")
            nc.vector.reciprocal(rag[:], zb[:])
            nc.vector.tensor_scalar_mul(age[:], age[:], rag[:])
            psq = psmm.tile([TA, 128], f32, tag="mm")
            nc.tensor.transpose(psq[:], age[:], idn[:])
            agT = iop.tile([TA, 128], f32, tag="agT")
            nc.scalar.copy(agT[:], psq[:])
            nc.sync.dma_start(agp_out[:, :], agT[:])

            # ---------- main: scores -> softmax rows, per 128-row strip ----------
            BCH = 2048                # ACT op width (4 PSUM banks)
            NB = N_TASK // BCH        # 8 big chunks per strip
            MM_PER_B = BCH // CHUNK   # 4 matmuls per big chunk
            for s in range(TL):
                zc = iop.tile([128, NB], f32, tag="zc")
                pieces = []
                for pc in range(NPIECE):
                    ep = epool.tile([128, PIECE], f32, tag="e")
                    pieces.append(ep)
                    for ci in range(PIECE // BCH):
                        ch = pc * (PIECE // BCH) + ci
                        ps = psmm.tile([128, BCH], f32, tag="mm")
                        # x[i, j] = s_task_hi[j] + s_task_lo[j]  (K=2 bf16)
                        for mi in range(MM_PER_B):
                            j0 = ch * BCH + mi * CHUNK
                            nc.tensor.matmul(
                                ps[:, mi * CHUNK:(mi + 1) * CHUNK],
                                ones2_bf[:],
                                rhs_bf[:, j0:j0 + CHUNK],
                                start=True, stop=True)
                        # t = leaky_relu(x + s_ag[i]) (ACT Prelu: bias per
                        # partition, alpha slope; same table set as Exp)
                        tscr = tscrp.tile([128, BCH], f32, tag="tscr")
                        nc.scalar.activation(tscr[:], ps[:], AF.Prelu,
                                             bias=sag_cols[:, s:s + 1], alpha=NEG)
                        # e = exp(t), accumulate row sums (ACT)
                        nc.scalar.activation(
                            ep[:, ci * BCH:(ci + 1) * BCH], tscr[:], AF.Exp,
                            accum_out=zc[:, ch:ch + 1])
                z = iop.tile([128, 1], f32, tag="z")
                nc.vector.tensor_reduce(z[:], zc[:], axis=mybir.AxisListType.X,
                                        op=ALU.add)
                r = iop.tile([128, 1], f32, tag="r")
                nc.vector.reciprocal(r[:], z[:])
                for pc in range(NPIECE):
                    nc.vector.tensor_scalar_mul(pieces[pc][:], pieces[pc][:], r[:])
                    nc.sync.dma_start(
                        policy_out[s * 128:(s + 1) * 128,
                                   pc * PIECE:(pc + 1) * PIECE],
                        pieces[pc][:])

    nc.compile()
    return nc


def _get_program():
    if "nc" not in _CACHE:
        _CACHE["nc"] = _build_program()
    return _CACHE["nc"]


def kernel(nf, w_att, w_ag, ag_node_indices, task_node_indices, task_finished,
           _want_trace=False):
    from concourse.bass_utils import run_bass_kernel_spmd

    nf = np.asarray(nf, dtype=np.float32)
    w_att = np.asarray(w_att, dtype=np.float32)
    w_ag = np.asarray(w_ag, dtype=np.float32)
    ag_idx = np.asarray(ag_node_indices).astype(np.int64)
    task_idx = np.asarray(task_node_indices).astype(np.int64)
    finished = np.asarray(task_finished).astype(bool)

    # host-side gather (index select) + shard prep
    ag_nfs = np.ascontiguousarray(nf[ag_idx])        # [2048, 128]
    task_nfs = np.ascontiguousarray(nf[task_idx])    # [16384, 128]
    w1 = w_att[:D, 0]
    w2 = w_att[D:, 0]
    wag = w_ag[:, 0]
    w1b = np.ascontiguousarray(np.broadcast_to(w1[None, :], (128, D)))
    w2b = np.ascontiguousarray(np.broadcast_to(w2[None, :], (128, D)))
    wagb = np.ascontiguousarray(np.broadcast_to(wag[None, :], (128, D)))
    ident = np.eye(128, dtype=np.float32)
    maskval = np.where(finished, np.float32(-1e30), np.float32(0)).astype(np.float32)

    in_maps = []
    for c in range(N_CORES):
        mv = maskval[c * T_LOC:(c + 1) * T_LOC].reshape(T_LOC // 128, 128)
        in_maps.append({
            "task_loc": np.ascontiguousarray(task_nfs[c * T_LOC:(c + 1) * T_LOC]),
            "ag_full": ag_nfs,
            "ag_loc": np.ascontiguousarray(ag_nfs[c * R:(c + 1) * R]),
            "w1b": w1b,
            "w2b": w2b,
            "wagb": wagb,
            "ident": ident,
            "mask_cols": np.ascontiguousarray(mv.T),
            "policy_out": np.zeros((R, N_TASK), dtype=np.float32),
            "agp_out": np.zeros((N_AG // 128, 128), dtype=np.float32),
        })

    nc = _get_program()
    res = run_bass_kernel_spmd(nc, in_maps, list(range(N_CORES)),
                               trace=_want_trace)

    policy = np.concatenate(
        [res.results[c]["policy_out"] for c in range(N_CORES)], axis=0)
    ag_policy = res.results[0]["agp_out"].reshape(N_AG)
    if _want_trace:
        _CACHE["last_results"] = res
    return policy, ag_policy


# revision 18
# speedup vs baseline: 1.0264x; 1.0264x over previous
"""Bass/Trainium2 kernel for nn_Bipartite_44014824849868 (GNN message passing).

Computes, for a full bipartite agent<-task graph:
  policy    = softmax_j( leaky_relu(ag_nfs @ w_att[D:] + task_nfs @ w_att[:D], 0.01) )  [n_ag, n_task]
  ag_policy = softmax( leaky_relu(ag_nfs @ w_ag, 0.01) )                                 [n_ag]

Sharding: agent (row) dimension split across 8 NeuronCores (256 rows each).
s_task = task_nfs @ w_att[:D] is computed sharded (2048 tasks per core) and
exchanged with one small AllGather; ag_policy is computed redundantly on
every core.
"""

import numpy as np

D = 128
N_AG = 2048
N_TASK = 16384
N_CORES = 8
R = N_AG // N_CORES          # 256 agent rows per core
T_LOC = N_TASK // N_CORES    # 2048 task rows per core
NEG = 0.01                   # leaky relu slope
CHUNK = 512                  # PSUM bank width in fp32
PIECE = 4096                 # output piece width (pipelining granularity)
NCH = N_TASK // CHUNK        # 32
NPIECE = N_TASK // PIECE     # 4
CH_PER_PIECE = PIECE // CHUNK

_CACHE = {}


def _build_program(_tlsim=False):
    import concourse.bacc as bacc
    import concourse.bass as bass
    import concourse.mybir as mybir
    from concourse import tile

    f32 = mybir.dt.float32
    AF = mybir.ActivationFunctionType
    ALU = mybir.AluOpType

    nc = bacc.Bacc("TRN2", target_bir_lowering=False, debug=False,
                   num_devices=1 if _tlsim else N_CORES)

    # ---- external I/O ----
    task_loc = nc.declare_dram_parameter("task_loc", [T_LOC, D], f32, isOutput=False)
    ag_full = nc.declare_dram_parameter("ag_full", [N_AG, D], f32, isOutput=False)
    ag_loc = nc.declare_dram_parameter("ag_loc", [R, D], f32, isOutput=False)
    w1b = nc.declare_dram_parameter("w1b", [128, D], f32, isOutput=False)
    w2b = nc.declare_dram_parameter("w2b", [128, D], f32, isOutput=False)
    wagb = nc.declare_dram_parameter("wagb", [128, D], f32, isOutput=False)
    ident = nc.declare_dram_parameter("ident", [128, 128], f32, isOutput=False)
    mask_cols = nc.declare_dram_parameter("mask_cols", [128, T_LOC // 128], f32,
                                          isOutput=False)
    policy_out = nc.declare_dram_parameter("policy_out", [R, N_TASK], f32,
                                           isOutput=True)
    agp_out = nc.declare_dram_parameter("agp_out", [N_AG // 128, 128], f32,
                                        isOutput=True)

    # ---- internal DRAM for the s_task exchange (bf16 hi/lo split) ----
    bf16 = mybir.dt.bfloat16
    st_loc_d = nc.dram_tensor("st_loc_d", [2, T_LOC // 128, 128], bf16)
    st_all_d = nc.dram_tensor("st_all_d", [N_CORES, 2, T_LOC], bf16,
                              addr_space="Shared")

    TT = T_LOC // 128   # 16 task tiles per core
    TA = N_AG // 128    # 16 agent tiles
    TL = R // 128       # 2 local agent tiles (= strips)

    with tile.TileContext(nc) as tc:
        with (
            tc.tile_pool(name="cpool", bufs=1) as cpool,
            tc.tile_pool(name="iop", bufs=2) as iop,
            tc.tile_pool(name="tscrp", bufs=3) as tscrp,
            tc.tile_pool(name="epool", bufs=5) as epool,
            tc.tile_pool(name="psmm", bufs=2, space=bass.MemorySpace.PSUM) as psmm,
        ):
            # ---------- input loads ----------
            task_sb = cpool.tile([128, TT, D], f32)
            task_view = task_loc[:, :].rearrange("(t p) d -> p t d", p=128)
            for q in range(4):
                nc.sync.dma_start(task_sb[:, q * (TT // 4):(q + 1) * (TT // 4), :],
                                  task_view[:, q * (TT // 4):(q + 1) * (TT // 4), :])
            w1s = cpool.tile([128, D], f32)
            nc.sync.dma_start(w1s[:], w1b[:, :])
            w2s = cpool.tile([128, D], f32)
            nc.sync.dma_start(w2s[:], w2b[:, :])
            wags = cpool.tile([128, D], f32)
            nc.sync.dma_start(wags[:], wagb[:, :])
            idn = cpool.tile([128, 128], f32)
            nc.sync.dma_start(idn[:], ident[:, :])
            maskc = cpool.tile([128, TT], f32)
            nc.sync.dma_start(maskc[:], mask_cols[:, :])

            # ---------- matvecs (DVE fused mult+reduce) ----------
            st_cols = cpool.tile([128, TT], f32)      # s_task local, col layout
            for t in range(TT):
                scr = iop.tile([128, D], f32, tag="scr")
                nc.vector.tensor_mul(scr[:], task_sb[:, t, :], w1s[:])
                nc.vector.tensor_reduce(st_cols[:, t:t + 1], scr[:],
                                        axis=mybir.AxisListType.X, op=ALU.add)
            # fold task_finished mask into s_task (adds -1e30 on finished cols)
            nc.vector.tensor_add(st_cols[:], st_cols[:], maskc[:])

            # ---------- s_task: cols -> row order, bf16 hi/lo, AllGather ----------
            pst = psmm.tile([TT, 128], f32, tag="mm")
            nc.tensor.transpose(pst[:], st_cols[:], idn[:])
            stT = cpool.tile([TT, 128], f32)
            nc.scalar.copy(stT[:], pst[:])
            # hi/lo split so the PE can consume s_task at bf16 speed with
            # ~fp32 accuracy: s_task ~= hi + lo, both bf16
            st_hi = cpool.tile([TT, 128], bf16)
            nc.vector.tensor_copy(st_hi[:], stT[:])
            st_hi32 = cpool.tile([TT, 128], f32)
            nc.vector.tensor_copy(st_hi32[:], st_hi[:])
            st_lo32 = cpool.tile([TT, 128], f32)
            nc.vector.tensor_sub(st_lo32[:], stT[:], st_hi32[:])
            st_lo = cpool.tile([TT, 128], bf16)
            nc.vector.tensor_copy(st_lo[:], st_lo32[:])
            nc.sync.dma_start(st_loc_d[0, :, :], st_hi[:])
            nc.sync.dma_start(st_loc_d[1, :, :], st_lo[:])
            if _tlsim:
                # stand-in for the AllGather so TimelineSim (single core,
                # no collectives) can model the rest of the kernel
                for c_ in range(N_CORES):
                    nc.sync.dma_start(
                        st_all_d[c_, :, :].rearrange("h (t p) -> h t p", p=128),
                        st_loc_d[:, :, :])
            else:
                nc.gpsimd.collective_compute(
                    "AllGather", ALU.bypass,
                    replica_groups=[list(range(N_CORES))],
                    ins=[st_loc_d[:, :, :].rearrange("a b c -> (a b c)")],
                    outs=[st_all_d[:, :, :].rearrange("a b c -> (a b c)")],
                )
            # gathered layout: [core, hi/lo, 2048] -> rhs rows [2, 16384] bf16
            # (partition 0 = s_task_hi, partition 1 = s_task_lo), one DMA
            rhs_bf = cpool.tile([2, N_TASK], bf16)
            nc.sync.dma_start(
                rhs_bf[:, :].rearrange("h (c j) -> h c j", c=N_CORES),
                st_all_d[:, :, :].rearrange("c h j -> h c j"))

            ag_sb = cpool.tile([128, TA, D], f32)
            nc.sync.dma_start(
                ag_sb[:], ag_full[:, :].rearrange("(t p) d -> p t d", p=128))
            agl_sb = cpool.tile([128, TL, D], f32)
            nc.sync.dma_start(
                agl_sb[:], ag_loc[:, :].rearrange("(t p) d -> p t d", p=128))

            sag_cols = cpool.tile([128, TL], f32)     # s_ag for local strips
            for t in range(TL):
                scr = iop.tile([128, D], f32, tag="scr")
                nc.vector.tensor_mul(scr[:], agl_sb[:, t, :], w2s[:])
                nc.vector.tensor_reduce(sag_cols[:, t:t + 1], scr[:],
                                        axis=mybir.AxisListType.X, op=ALU.add)

            ags_cols = cpool.tile([128, TA], f32)     # ag_score pre-act, all agents
            for t in range(TA):
                scr = iop.tile([128, D], f32, tag="scr")
                nc.vector.tensor_mul(scr[:], ag_sb[:, t, :], wags[:])
                nc.vector.tensor_reduce(ags_cols[:, t:t + 1], scr[:],
                                        axis=mybir.AxisListType.X, op=ALU.add)

            ones2_bf = cpool.tile([2, 128], bf16)
            nc.vector.memset(ones2_bf[:], 1.0)
            ones_r = cpool.tile([1, 128], f32)
            nc.vector.memset(ones_r[:], 1.0)
            ones_c = cpool.tile([128, 1], f32)
            nc.vector.memset(ones_c[:], 1.0)



            # ---------- ag_policy (redundant on every core) ----------
            agt = iop.tile([128, TA], f32, tag="agt")
            nc.scalar.activation(agt[:], ags_cols[:], AF.Prelu, alpha=NEG)
            age = iop.tile([128, TA], f32, tag="age")
            agz = iop.tile([128, 1], f32, tag="agz")
            nc.scalar.activation(age[:], agt[:], AF.Exp, accum_out=agz[:])
            psz = psmm.tile([1, 1], f32, tag="mm")
            nc.tensor.matmul(psz[:], agz[:], ones_c[:], start=True, stop=True)
            zz = iop.tile([1, 1], f32, tag="zz")
            nc.scalar.copy(zz[:], psz[:])
            psb = psmm.tile([128, 1], f32, tag="mm")
            nc.tensor.matmul(psb[:], ones_r[:], zz[:], start=True, stop=True)
            zb = iop.tile([128, 1], f32, tag="zb")
            nc.scalar.copy(zb[:], psb[:])
            rag = iop.tile([128, 1], f32, tag="rag")
            nc.vector.reciprocal(rag[:], zb[:])
            nc.vector.tensor_scalar_mul(age[:], age[:], rag[:])
            psq = psmm.tile([TA, 128], f32, tag="mm")
            nc.tensor.transpose(psq[:], age[:], idn[:])
            agT = iop.tile([TA, 128], f32, tag="agT")
            nc.scalar.copy(agT[:], psq[:])
            nc.sync.dma_start(agp_out[:, :], agT[:])

            # ---------- main: scores -> softmax rows, per 128-row strip ----------
            BCH = 2048                # ACT op width (4 PSUM banks)
            NB = N_TASK // BCH        # 8 big chunks per strip
            MM_PER_B = BCH // CHUNK   # 4 matmuls per big chunk
            for s in range(TL):
                zc = iop.tile([128, NB], f32, tag="zc")
                pieces = []
                for pc in range(NPIECE):
                    ep = epool.tile([128, PIECE], f32, tag="e")
                    pieces.append(ep)
                    for ci in range(PIECE // BCH):
                        ch = pc * (PIECE // BCH) + ci
                        ps = psmm.tile([128, BCH], f32, tag="mm")
                        # x[i, j] = s_task_hi[j] + s_task_lo[j]  (K=2 bf16)
                        for mi in range(MM_PER_B):
                            j0 = ch * BCH + mi * CHUNK
                            nc.tensor.matmul(
                                ps[:, mi * CHUNK:(mi + 1) * CHUNK],
                                ones2_bf[:],
                                rhs_bf[:, j0:j0 + CHUNK],
                                start=True, stop=True)
                        # t = leaky_relu(x + s_ag[i]) (ACT Prelu: bias per
                        # partition, alpha slope; same table set as Exp)
                        tscr = tscrp.tile([128, BCH], f32, tag="tscr")
                        nc.scalar.activation(tscr[:], ps[:], AF.Prelu,
                                             bias=sag_cols[:, s:s + 1], alpha=NEG)
                        # e = exp(t), accumulate row sums (ACT)
                        nc.scalar.activation(
                            ep[:, ci * BCH:(ci + 1) * BCH], tscr[:], AF.Exp,
                            accum_out=zc[:, ch:ch + 1])
                z = iop.tile([128, 1], f32, tag="z")
                nc.vector.tensor_reduce(z[:], zc[:], axis=mybir.AxisListType.X,
                                        op=ALU.add)
                r = iop.tile([128, 1], f32, tag="r")
                nc.vector.reciprocal(r[:], z[:])
                for pc in range(NPIECE):
                    nc.vector.tensor_scalar_mul(pieces[pc][:], pieces[pc][:], r[:])
                    nc.sync.dma_start(
                        policy_out[s * 128:(s + 1) * 128,
                                   pc * PIECE:(pc + 1) * PIECE],
                        pieces[pc][:])

    nc.compile()
    return nc


def _get_program():
    if "nc" not in _CACHE:
        _CACHE["nc"] = _build_program()
    return _CACHE["nc"]


def kernel(nf, w_att, w_ag, ag_node_indices, task_node_indices, task_finished,
           _want_trace=False):
    from concourse.bass_utils import run_bass_kernel_spmd

    nf = np.asarray(nf, dtype=np.float32)
    w_att = np.asarray(w_att, dtype=np.float32)
    w_ag = np.asarray(w_ag, dtype=np.float32)
    ag_idx = np.asarray(ag_node_indices).astype(np.int64)
    task_idx = np.asarray(task_node_indices).astype(np.int64)
    finished = np.asarray(task_finished).astype(bool)

    # host-side gather (index select) + shard prep
    ag_nfs = np.ascontiguousarray(nf[ag_idx])        # [2048, 128]
    task_nfs = np.ascontiguousarray(nf[task_idx])    # [16384, 128]
    w1 = w_att[:D, 0]
    w2 = w_att[D:, 0]
    wag = w_ag[:, 0]
    w1b = np.ascontiguousarray(np.broadcast_to(w1[None, :], (128, D)))
    w2b = np.ascontiguousarray(np.broadcast_to(w2[None, :], (128, D)))
    wagb = np.ascontiguousarray(np.broadcast_to(wag[None, :], (128, D)))
    ident = np.eye(128, dtype=np.float32)
    maskval = np.where(finished, np.float32(-1e30), np.float32(0)).astype(np.float32)

    in_maps = []
    for c in range(N_CORES):
        mv = maskval[c * T_LOC:(c + 1) * T_LOC].reshape(T_LOC // 128, 128)
        in_maps.append({
            "task_loc": np.ascontiguousarray(task_nfs[c * T_LOC:(c + 1) * T_LOC]),
            "ag_full": ag_nfs,
            "ag_loc": np.ascontiguousarray(ag_nfs[c * R:(c + 1) * R]),
            "w1b": w1b,
            "w2b": w2b,
            "wagb": wagb,
            "ident": ident,
            "mask_cols": np.ascontiguousarray(mv.T),
            "policy_out": np.zeros((R, N_TASK), dtype=np.float32),
            "agp_out": np.zeros((N_AG // 128, 128), dtype=np.float32),
        })

    nc = _get_program()
    res = run_bass_kernel_spmd(nc, in_maps, list(range(N_CORES)),
                               trace=_want_trace)

    policy = np.concatenate(
        [res.results[c]["policy_out"] for c in range(N_CORES)], axis=0)
    ag_policy = res.results[0]["agp_out"].reshape(N_AG)
    if _want_trace:
        _CACHE["last_results"] = res
    return policy, ag_policy


# revision 19
# speedup vs baseline: 1.0657x; 1.0382x over previous
"""Bass/Trainium2 kernel for nn_Bipartite_44014824849868 (GNN message passing).

Computes, for a full bipartite agent<-task graph:
  policy    = softmax_j( leaky_relu(ag_nfs @ w_att[D:] + task_nfs @ w_att[:D], 0.01) )  [n_ag, n_task]
  ag_policy = softmax( leaky_relu(ag_nfs @ w_ag, 0.01) )                                 [n_ag]

Sharding: agent (row) dimension split across 8 NeuronCores (256 rows each).
s_task = task_nfs @ w_att[:D] is computed sharded (2048 tasks per core) and
exchanged with one small AllGather; ag_policy is computed redundantly on
every core.
"""

import numpy as np

D = 128
N_AG = 2048
N_TASK = 16384
N_CORES = 8
R = N_AG // N_CORES          # 256 agent rows per core
T_LOC = N_TASK // N_CORES    # 2048 task rows per core
NEG = 0.01                   # leaky relu slope
CHUNK = 512                  # PSUM bank width in fp32
PIECE = 4096                 # output piece width (pipelining granularity)
NCH = N_TASK // CHUNK        # 32
NPIECE = N_TASK // PIECE     # 4
CH_PER_PIECE = PIECE // CHUNK

_CACHE = {}


def _build_program(_tlsim=False):
    import concourse.bacc as bacc
    import concourse.bass as bass
    import concourse.mybir as mybir
    from concourse import tile

    f32 = mybir.dt.float32
    AF = mybir.ActivationFunctionType
    ALU = mybir.AluOpType

    nc = bacc.Bacc("TRN2", target_bir_lowering=False, debug=False,
                   num_devices=1 if _tlsim else N_CORES)

    # ---- external I/O ----
    task_loc = nc.declare_dram_parameter("task_loc", [T_LOC, D], f32, isOutput=False)
    ag_full = nc.declare_dram_parameter("ag_full", [N_AG, D], f32, isOutput=False)
    ag_loc = nc.declare_dram_parameter("ag_loc", [R, D], f32, isOutput=False)
    w1b = nc.declare_dram_parameter("w1b", [128, D], f32, isOutput=False)
    w2b = nc.declare_dram_parameter("w2b", [128, D], f32, isOutput=False)
    wagb = nc.declare_dram_parameter("wagb", [128, D], f32, isOutput=False)
    ident = nc.declare_dram_parameter("ident", [128, 128], f32, isOutput=False)
    mask_cols = nc.declare_dram_parameter("mask_cols", [128, T_LOC // 128], f32,
                                          isOutput=False)
    policy_out = nc.declare_dram_parameter("policy_out", [R, N_TASK], f32,
                                           isOutput=True)
    agp_out = nc.declare_dram_parameter("agp_out", [N_AG // 128, 128], f32,
                                        isOutput=True)

    # ---- internal DRAM for the s_task exchange (bf16 hi/lo split) ----
    bf16 = mybir.dt.bfloat16
    st_loc_d = nc.dram_tensor("st_loc_d", [2, T_LOC // 128, 128], bf16)
    st_all_d = nc.dram_tensor("st_all_d", [N_CORES, 2, T_LOC], bf16,
                              addr_space="Shared")

    TT = T_LOC // 128   # 16 task tiles per core
    TA = N_AG // 128    # 16 agent tiles
    TL = R // 128       # 2 local agent tiles (= strips)

    with tile.TileContext(nc) as tc:
        with (
            tc.tile_pool(name="cpool", bufs=1) as cpool,
            tc.tile_pool(name="iop", bufs=2) as iop,
            tc.tile_pool(name="tscrp", bufs=2) as tscrp,
            tc.tile_pool(name="epool", bufs=5) as epool,
            tc.tile_pool(name="psmm", bufs=2, space=bass.MemorySpace.PSUM) as psmm,
        ):
            # ---------- input loads ----------
            task_sb = cpool.tile([128, TT, D], f32)
            task_view = task_loc[:, :].rearrange("(t p) d -> p t d", p=128)
            for q in range(4):
                nc.sync.dma_start(task_sb[:, q * (TT // 4):(q + 1) * (TT // 4), :],
                                  task_view[:, q * (TT // 4):(q + 1) * (TT // 4), :])
            w1s = cpool.tile([128, D], f32)
            nc.sync.dma_start(w1s[:], w1b[:, :])
            w2s = cpool.tile([128, D], f32)
            nc.sync.dma_start(w2s[:], w2b[:, :])
            wags = cpool.tile([128, D], f32)
            nc.sync.dma_start(wags[:], wagb[:, :])
            idn = cpool.tile([128, 128], f32)
            nc.sync.dma_start(idn[:], ident[:, :])
            maskc = cpool.tile([128, TT], f32)
            nc.sync.dma_start(maskc[:], mask_cols[:, :])

            # ---------- matvecs (DVE fused mult+reduce) ----------
            st_cols = cpool.tile([128, TT], f32)      # s_task local, col layout
            for t in range(TT):
                scr = iop.tile([128, D], f32, tag="scr")
                nc.vector.tensor_mul(scr[:], task_sb[:, t, :], w1s[:])
                nc.vector.tensor_reduce(st_cols[:, t:t + 1], scr[:],
                                        axis=mybir.AxisListType.X, op=ALU.add)
            # fold task_finished mask into s_task (adds -1e30 on finished cols)
            nc.vector.tensor_add(st_cols[:], st_cols[:], maskc[:])

            # ---------- s_task: cols -> row order, bf16 hi/lo, AllGather ----------
            pst = psmm.tile([TT, 128], f32, tag="mm")
            nc.tensor.transpose(pst[:], st_cols[:], idn[:])
            # hi/lo split so the PE can consume s_task at bf16 speed with
            # ~fp32 accuracy: s_task ~= hi + lo, both bf16 (each DVE op
            # reads the PSUM transpose at most once)
            st_hi = cpool.tile([TT, 128], bf16)
            nc.vector.tensor_copy(st_hi[:], pst[:])
            st_hi32 = cpool.tile([TT, 128], f32)
            nc.vector.tensor_copy(st_hi32[:], st_hi[:])
            st_lo32 = cpool.tile([TT, 128], f32)
            nc.vector.tensor_sub(st_lo32[:], pst[:], st_hi32[:])
            st_lo = cpool.tile([TT, 128], bf16)
            nc.vector.tensor_copy(st_lo[:], st_lo32[:])
            nc.sync.dma_start(st_loc_d[0, :, :], st_hi[:])
            nc.sync.dma_start(st_loc_d[1, :, :], st_lo[:])
            if _tlsim:
                # stand-in for the AllGather so TimelineSim (single core,
                # no collectives) can model the rest of the kernel
                for c_ in range(N_CORES):
                    nc.sync.dma_start(
                        st_all_d[c_, :, :].rearrange("h (t p) -> h t p", p=128),
                        st_loc_d[:, :, :])
            else:
                nc.gpsimd.collective_compute(
                    "AllGather", ALU.bypass,
                    replica_groups=[list(range(N_CORES))],
                    ins=[st_loc_d[:, :, :].rearrange("a b c -> (a b c)")],
                    outs=[st_all_d[:, :, :].rearrange("a b c -> (a b c)")],
                )
            # gathered layout: [core, hi/lo, 2048] -> rhs rows [2, 16384] bf16
            # (partition 0 = s_task_hi, partition 1 = s_task_lo), one DMA
            rhs_bf = cpool.tile([2, N_TASK], bf16)
            nc.sync.dma_start(
                rhs_bf[:, :].rearrange("h (c j) -> h c j", c=N_CORES),
                st_all_d[:, :, :].rearrange("c h j -> h c j"))

            ag_sb = cpool.tile([128, TA, D], f32)
            nc.sync.dma_start(
                ag_sb[:], ag_full[:, :].rearrange("(t p) d -> p t d", p=128))
            agl_sb = cpool.tile([128, TL, D], f32)
            nc.sync.dma_start(
                agl_sb[:], ag_loc[:, :].rearrange("(t p) d -> p t d", p=128))

            sag_cols = cpool.tile([128, TL], f32)     # s_ag for local strips
            for t in range(TL):
                scr = iop.tile([128, D], f32, tag="scr")
                nc.vector.tensor_mul(scr[:], agl_sb[:, t, :], w2s[:])
                nc.vector.tensor_reduce(sag_cols[:, t:t + 1], scr[:],
                                        axis=mybir.AxisListType.X, op=ALU.add)

            ags_cols = cpool.tile([128, TA], f32)     # ag_score pre-act, all agents
            for t in range(TA):
                scr = iop.tile([128, D], f32, tag="scr")
                nc.vector.tensor_mul(scr[:], ag_sb[:, t, :], wags[:])
                nc.vector.tensor_reduce(ags_cols[:, t:t + 1], scr[:],
                                        axis=mybir.AxisListType.X, op=ALU.add)

            ones2_bf = cpool.tile([2, 128], bf16)
            nc.vector.memset(ones2_bf[:], 1.0)
            ones_r = cpool.tile([1, 128], f32)
            nc.vector.memset(ones_r[:], 1.0)
            ones_c = cpool.tile([128, 1], f32)
            nc.vector.memset(ones_c[:], 1.0)



            # ---------- ag_policy (redundant on every core) ----------
            agt = iop.tile([128, TA], f32, tag="agt")
            nc.scalar.activation(agt[:], ags_cols[:], AF.Prelu, alpha=NEG)
            age = iop.tile([128, TA], f32, tag="age")
            agz = iop.tile([128, 1], f32, tag="agz")
            nc.scalar.activation(age[:], agt[:], AF.Exp, accum_out=agz[:])
            psz = psmm.tile([1, 1], f32, tag="mm")
            nc.tensor.matmul(psz[:], agz[:], ones_c[:], start=True, stop=True)
            zz = iop.tile([1, 1], f32, tag="zz")
            nc.scalar.copy(zz[:], psz[:])
            psb = psmm.tile([128, 1], f32, tag="mm")
            nc.tensor.matmul(psb[:], ones_r[:], zz[:], start=True, stop=True)
            zb = iop.tile([128, 1], f32, tag="zb")
            nc.scalar.copy(zb[:], psb[:])
            rag = iop.tile([128, 1], f32, tag="rag")
            nc.vector.reciprocal(rag[:], zb[:])
            nc.vector.tensor_scalar_mul(age[:], age[:], rag[:])
            psq = psmm.tile([TA, 128], f32, tag="mm")
            nc.tensor.transpose(psq[:], age[:], idn[:])
            agT = iop.tile([TA, 128], f32, tag="agT")
            nc.scalar.copy(agT[:], psq[:])
            nc.sync.dma_start(agp_out[:, :], agT[:])

            # ---------- main: scores -> softmax rows, per 128-row strip ----------
            BCH = 2048                # ACT op width (4 PSUM banks)
            NB = N_TASK // BCH        # 8 big chunks per strip
            MM_PER_B = BCH // CHUNK   # 4 matmuls per big chunk
            for s in range(TL):
                zc = iop.tile([128, NPIECE], f32, tag="zc")
                pieces = []
                for pc in range(NPIECE):
                    ep = epool.tile([128, PIECE], f32, tag="e")
                    pieces.append(ep)
                    tscr = tscrp.tile([128, PIECE], f32, tag="tscr")
                    for ci in range(PIECE // BCH):
                        ch = pc * (PIECE // BCH) + ci
                        ps = psmm.tile([128, BCH], f32, tag="mm")
                        # x[i, j] = s_task_hi[j] + s_task_lo[j]  (K=2 bf16)
                        for mi in range(MM_PER_B):
                            j0 = ch * BCH + mi * CHUNK
                            nc.tensor.matmul(
                                ps[:, mi * CHUNK:(mi + 1) * CHUNK],
                                ones2_bf[:],
                                rhs_bf[:, j0:j0 + CHUNK],
                                start=True, stop=True)
                        # t = leaky_relu(x + s_ag[i]) (ACT Prelu: bias per
                        # partition, alpha slope; same table set as Exp)
                        nc.scalar.activation(tscr[:, ci * BCH:(ci + 1) * BCH],
                                             ps[:], AF.Prelu,
                                             bias=sag_cols[:, s:s + 1], alpha=NEG)
                    # e = exp(t) over the whole 4096 piece (SBUF source, so
                    # not PSUM-bank-limited), accumulate row sums (ACT)
                    nc.scalar.activation(
                        ep[:], tscr[:], AF.Exp,
                        accum_out=zc[:, pc:pc + 1])
                z = iop.tile([128, 1], f32, tag="z")
                nc.vector.tensor_reduce(z[:], zc[:], axis=mybir.AxisListType.X,
                                        op=ALU.add)
                r = iop.tile([128, 1], f32, tag="r")
                nc.vector.reciprocal(r[:], z[:])
                for pc in range(NPIECE):
                    nc.vector.tensor_scalar_mul(pieces[pc][:], pieces[pc][:], r[:])
                    nc.sync.dma_start(
                        policy_out[s * 128:(s + 1) * 128,
                                   pc * PIECE:(pc + 1) * PIECE],
                        pieces[pc][:])

    nc.compile()
    return nc


def _get_program():
    if "nc" not in _CACHE:
        _CACHE["nc"] = _build_program()
    return _CACHE["nc"]


def kernel(nf, w_att, w_ag, ag_node_indices, task_node_indices, task_finished,
           _want_trace=False):
    from concourse.bass_utils import run_bass_kernel_spmd

    nf = np.asarray(nf, dtype=np.float32)
    w_att = np.asarray(w_att, dtype=np.float32)
    w_ag = np.asarray(w_ag, dtype=np.float32)
    ag_idx = np.asarray(ag_node_indices).astype(np.int64)
    task_idx = np.asarray(task_node_indices).astype(np.int64)
    finished = np.asarray(task_finished).astype(bool)

    # host-side gather (index select) + shard prep
    ag_nfs = np.ascontiguousarray(nf[ag_idx])        # [2048, 128]
    task_nfs = np.ascontiguousarray(nf[task_idx])    # [16384, 128]
    w1 = w_att[:D, 0]
    w2 = w_att[D:, 0]
    wag = w_ag[:, 0]
    w1b = np.ascontiguousarray(np.broadcast_to(w1[None, :], (128, D)))
    w2b = np.ascontiguousarray(np.broadcast_to(w2[None, :], (128, D)))
    wagb = np.ascontiguousarray(np.broadcast_to(wag[None, :], (128, D)))
    ident = np.eye(128, dtype=np.float32)
    maskval = np.where(finished, np.float32(-1e30), np.float32(0)).astype(np.float32)

    in_maps = []
    for c in range(N_CORES):
        mv = maskval[c * T_LOC:(c + 1) * T_LOC].reshape(T_LOC // 128, 128)
        in_maps.append({
            "task_loc": np.ascontiguousarray(task_nfs[c * T_LOC:(c + 1) * T_LOC]),
            "ag_full": ag_nfs,
            "ag_loc": np.ascontiguousarray(ag_nfs[c * R:(c + 1) * R]),
            "w1b": w1b,
            "w2b": w2b,
            "wagb": wagb,
            "ident": ident,
            "mask_cols": np.ascontiguousarray(mv.T),
            "policy_out": np.zeros((R, N_TASK), dtype=np.float32),
            "agp_out": np.zeros((N_AG // 128, 128), dtype=np.float32),
        })

    nc = _get_program()
    res = run_bass_kernel_spmd(nc, in_maps, list(range(N_CORES)),
                               trace=_want_trace)

    policy = np.concatenate(
        [res.results[c]["policy_out"] for c in range(N_CORES)], axis=0)
    ag_policy = res.results[0]["agp_out"].reshape(N_AG)
    if _want_trace:
        _CACHE["last_results"] = res
    return policy, ag_policy
